# revision 6
# baseline (speedup 1.0000x reference)
"""MoE routing kernel for Trainium2, expert-parallel across 8 NeuronCores.

Strategy (matches the expert-parallel sharding hint):
  Launch 1 (token-parallel gating): each core computes logits = gate_x_shard
    @ w_gate for its 512-token shard, top-4 mask + softmax -> dense sparse
    gates [512, 16], plus per-expert importance/load partials, AllReduce of
    the partials across the 8 cores and the cv^2 aux loss on-device.
  Host dispatch: tokens are gathered per expert from the device-computed
    gates (pure index/data movement), padded to a fixed capacity C.
  Launch 2 (expert-parallel MLP): core c owns experts {2c, 2c+1}. For each:
    h = relu(xg @ w1[e] + b1[e]); out = (h @ w2[e] + b2[e]) * gate, all
    matmuls on the PE array in fp32r (full rate), biases fused (b1 via the
    ACT bias port, b2 via a K=1 ones-row matmul into the same PSUM group).
  Host combine: out rows scatter-add back to token order (unique indices
    per expert, so a plain fancy-index += per expert).
"""

import functools

import numpy as np

import concourse.bass as bass
import concourse.mybir as mybir
from concourse.bass_utils import run_bass_kernel_spmd
from concourse.tile import TileContext
from concourse.vector_clock import ScopedClock, VectorClock

F32 = mybir.dt.float32
F32R = mybir.dt.float32r
AX = mybir.AxisListType
ALU = mybir.AluOpType
ACTF = mybir.ActivationFunctionType

B, D, HD, E, TOPK = 4096, 512, 1024, 16, 4
NCORES = 8
P = 128
BSH = B // NCORES  # 512 gating tokens per core
EPC = E // NCORES  # 2 experts per core
KD = D // P        # 4 contraction slices over D
KH = HD // P       # 8 slices over H
CORE_IDS = list(range(NCORES))


class _TileContextNX(TileContext):
    pass


def _legalize_waits(nc):
    """The AWS neuronxcc CoreV3 codegen accepts only one sync-wait per
    instruction, while Tile attaches all required waits to the consuming
    instruction. Hoist extra waits onto standalone same-engine NoOps placed
    immediately before the instruction (engine queues are FIFO, so ordering
    is preserved)."""
    n = 0
    for f in nc.m.functions:
        for bb in f.blocks:
            new = []
            for inst in bb.instructions:
                si = inst.sync_info
                if si is not None and si.on_wait and len(si.on_wait) > 1:
                    waits = list(si.on_wait)
                    for w in waits[:-1]:
                        n += 1
                        nop = mybir.InstNoOp(
                            name=f"{inst.name}-lw{n}",
                            engine=inst.engine,
                            sync_info=mybir.SyncInfo(on_wait=[w], on_update=[]),
                        )
                        new.append(nop)
                    si.on_wait = [waits[-1]]
                new.append(inst)
            bb.instructions[:] = new
    return nc


def _r(ap):
    """View an fp32 AP as fp32r so the PE runs at full (1 cyc/row) rate."""
    return ap.bitcast(F32R)


@functools.lru_cache(maxsize=None)
def _gating_nc():
    nc = bass.Bass("TRN2", target_bir_lowering=False, debug=False)
    gxT = nc.declare_dram_parameter("gxT", [D, BSH], F32, isOutput=False)
    wg = nc.declare_dram_parameter("wg", [D, E], F32, isOutput=False)
    gates_out = nc.declare_dram_parameter("gates", [BSH, E], F32, isOutput=True)
    loss_out = nc.declare_dram_parameter("loss", [1, 1], F32, isOutput=True)

    with _TileContextNX(nc) as tc:
        with (
            tc.tile_pool(name="sb", bufs=2) as sb,
            tc.tile_pool(name="acc", bufs=1) as acc,
            tc.tile_pool(name="psl", bufs=2, space="PSUM") as psl,
            tc.tile_pool(name="psa", bufs=1, space="PSUM") as psa,
            tc.tile_pool(name="dram", bufs=1, space="DRAM") as dram,
        ):
            wg_sb = acc.tile([P, KD, E], F32, name="wg", tag="wg")
            nc.sync.dma_start(wg_sb[:], wg.rearrange("(o p) e -> p o e", p=P))
            gx_sb = acc.tile([P, KD, BSH], F32, name="gx", tag="gx")
            for ks in range(KD):
                nc.sync.dma_start(gx_sb[:, ks, :], gxT[ks * P:(ks + 1) * P, :])
            ones_sb = acc.tile([P, 1], F32, name="ones", tag="ones")
            nc.vector.memset(ones_sb[:], 1.0)

            ps_imp = psa.tile([E, 1], F32, name="imp", tag="imp")
            ps_load = psa.tile([E, 1], F32, name="load", tag="load")

            ntt = BSH // P  # 4 token tiles
            for ti in range(ntt):
                ps = psl.tile([P, E], F32, name="logits", tag="logits")
                for ks in range(KD):
                    # full fp32: the top-4 boundary must match the fp32
                    # reference (smallest observed top4/5 gap ~2.5e-6)
                    nc.tensor.matmul(
                        ps[:],
                        lhsT=gx_sb[:, ks, ti * P:(ti + 1) * P],
                        rhs=wg_sb[:, ks, :],
                        start=(ks == 0),
                        stop=(ks == KD - 1),
                    )
                logits = sb.tile([P, E], F32, name="logits_sb", tag="logits_sb")
                nc.vector.tensor_copy(logits[:], ps[:])
                mx = sb.tile([P, 8], F32, name="mx", tag="mx")
                nc.vector.max(mx[:], logits[:])
                neg = sb.tile([P, 1], F32, name="neg", tag="neg")
                nc.vector.tensor_scalar_mul(neg[:], mx[:, 0:1], -1.0)
                expz = sb.tile([P, E], F32, name="expz", tag="expz")
                nc.scalar.activation(expz[:], logits[:], ACTF.Exp, bias=neg[:])
                mask = sb.tile([P, E], F32, name="mask", tag="mask")
                nc.vector.tensor_scalar(
                    mask[:], logits[:], mx[:, 3:4], None, op0=ALU.is_ge
                )
                nc.vector.tensor_mul(expz[:], expz[:], mask[:])
                s = sb.tile([P, 1], F32, name="s", tag="s")
                nc.vector.reduce_sum(s[:], expz[:], axis=AX.X)
                nc.vector.reciprocal(s[:], s[:])
                gates_sb = sb.tile([P, E], F32, name="gates_sb", tag="gates_sb")
                nc.vector.tensor_scalar_mul(gates_sb[:], expz[:], s[:])
                nc.sync.dma_start(gates_out[ti * P:(ti + 1) * P, :], gates_sb[:])

                gtz = sb.tile([P, E], F32, name="gtz", tag="gtz")
                nc.vector.tensor_scalar(
                    gtz[:], gates_sb[:], 0.0, None, op0=ALU.is_gt
                )
                nc.tensor.matmul(
                    ps_imp[:], lhsT=gates_sb[:], rhs=ones_sb[:],
                    start=(ti == 0), stop=(ti == ntt - 1),
                )
                nc.tensor.matmul(
                    ps_load[:], lhsT=gtz[:], rhs=ones_sb[:],
                    start=(ti == 0), stop=(ti == ntt - 1),
                )

            stat_sb = sb.tile([E, 2], F32, name="stat", tag="stat")
            nc.vector.tensor_copy(stat_sb[:, 0:1], ps_imp[:])
            nc.vector.tensor_copy(stat_sb[:, 1:2], ps_load[:])
            part_d = dram.tile([E, 2], F32, name="part", tag="part")
            red_d = dram.tile([E, 2], F32, name="red", tag="red")
            nc.sync.dma_start(part_d[:], stat_sb[:])
            nc.gpsimd.collective_compute(
                "AllReduce",
                ALU.add,
                replica_groups=[CORE_IDS],
                ins=[part_d.opt()],
                outs=[red_d.opt()],
            )
            red_sb = sb.tile([1, E, 2], F32, name="red_sb", tag="red_sb")
            nc.sync.dma_start(red_sb[:], red_d[None, :, :])

            def _t11(tag):
                return sb.tile([1, 1], F32, name=tag, tag=tag)

            cvs = []
            for si in range(2):
                v = red_sb[:, :, si]  # [1, E]
                sm = _t11(f"sm{si}")
                nc.vector.reduce_sum(sm[:], v, axis=AX.X)
                sq = sb.tile([1, E], F32, name=f"sq{si}", tag=f"sq{si}")
                nc.vector.tensor_mul(sq[:], v, v)
                ssq = _t11(f"ssq{si}")
                nc.vector.reduce_sum(ssq[:], sq[:], axis=AX.X)
                mean = _t11(f"mean{si}")
                nc.vector.tensor_scalar_mul(mean[:], sm[:], 1.0 / E)
                m2 = _t11(f"m2{si}")
                nc.vector.tensor_mul(m2[:], mean[:], mean[:])
                nm2 = _t11(f"nm2{si}")
                nc.vector.tensor_scalar_mul(nm2[:], m2[:], -float(E))
                var = _t11(f"var{si}")
                nc.vector.tensor_add(var[:], ssq[:], nm2[:])
                nc.vector.tensor_scalar_mul(var[:], var[:], 1.0 / (E - 1))
                den = _t11(f"den{si}")
                nc.vector.tensor_scalar_add(den[:], m2[:], 1e-10)
                nc.vector.reciprocal(den[:], den[:])
                cv = _t11(f"cv{si}")
                nc.vector.tensor_mul(cv[:], var[:], den[:])
                cvs.append(cv)
            loss_sb = _t11("loss_sb")
            nc.vector.tensor_add(loss_sb[:], cvs[0][:], cvs[1][:])
            nc.sync.dma_start(loss_out[:], loss_sb[:])
    return _legalize_waits(nc)


@functools.lru_cache(maxsize=None)
def _expert_nc(C):
    assert C % P == 0
    NT = C // P                      # token tiles per expert
    chunks = []
    off = 0
    while off < C:
        chunks.append((off, min(512, C - off)))
        off += 512

    nc = bass.Bass("TRN2", target_bir_lowering=False, debug=False)
    xgT = nc.declare_dram_parameter("xgT", [EPC, D, C], F32R, isOutput=False)
    w1 = nc.declare_dram_parameter("w1", [EPC, D, HD], F32R, isOutput=False)
    b1 = nc.declare_dram_parameter("b1", [EPC, HD], F32, isOutput=False)
    w2 = nc.declare_dram_parameter("w2", [EPC, HD, D], F32R, isOutput=False)
    b2 = nc.declare_dram_parameter("b2", [EPC, D], F32R, isOutput=False)
    g = nc.declare_dram_parameter("g", [EPC, C], F32, isOutput=False)
    ones = nc.declare_dram_parameter("ones", [1, P], F32R, isOutput=False)
    out = nc.declare_dram_parameter("out", [EPC, C, D], F32, isOutput=True)

    with _TileContextNX(nc) as tc:
        with (
            tc.tile_pool(name="wts", bufs=2) as wts,
            tc.tile_pool(name="xg", bufs=2) as xgp,
            tc.tile_pool(name="hb", bufs=2) as hp,
            tc.tile_pool(name="small", bufs=2) as small,
            tc.tile_pool(name="ob", bufs=3) as ob,
            tc.tile_pool(name="ps1", bufs=2, space="PSUM") as ps1p,
            tc.tile_pool(name="ps2", bufs=2, space="PSUM") as ps2p,
        ):
            ones1 = small.tile([1, P], F32R, name="ones1", tag="ones1")
            nc.sync.dma_start(ones1[:], ones[:])
            for j in range(EPC):
                w1_sb = wts.tile([P, KD, HD], F32R, name="w1", tag="w1")
                xg_sb = xgp.tile([P, KD, C], F32R, name="xg", tag="xg")
                for ks in range(KD):
                    nc.sync.dma_start(w1_sb[:, ks, :], w1[j, ks * P:(ks + 1) * P, :])
                    nc.sync.dma_start(xg_sb[:, ks, :], xgT[j, ks * P:(ks + 1) * P, :])
                w2_sb = wts.tile([P, KH, D], F32R, name="w2", tag="w2")
                for ht in range(KH):
                    nc.sync.dma_start(w2_sb[:, ht, :], w2[j, ht * P:(ht + 1) * P, :])
                b1_sb = small.tile([P, KH], F32, name="b1", tag="b1")
                nc.sync.dma_start(b1_sb[:], b1[j].rearrange("(o p) -> p o", p=P))
                b2_sb = small.tile([1, D], F32R, name="b2", tag="b2")
                nc.sync.dma_start(b2_sb[:], b2[j][None, :])
                g_sb = small.tile([P, NT], F32, name="g", tag="g")
                nc.sync.dma_start(g_sb[:], g[j].rearrange("(o p) -> p o", p=P))

                # layer 1: hT[h, c] = relu(w1.T @ xg + b1), H on partitions
                h_sb = hp.tile([P, KH, C], F32R, name="h", tag="h")
                for ht in range(KH):
                    pss = [
                        ps1p.tile([P, 512], F32, name=f"ps1_{ci}", tag=f"ps1_{ci}")
                        for ci in range(len(chunks))
                    ]
                    for ks in range(KD):
                        for ci, (co, cs) in enumerate(chunks):
                            nc.tensor.matmul(
                                pss[ci][:, :cs],
                                lhsT=w1_sb[:, ks, ht * P:(ht + 1) * P],
                                rhs=xg_sb[:, ks, co:co + cs],
                                start=(ks == 0),
                                stop=(ks == KD - 1),
                            )
                    for ci, (co, cs) in enumerate(chunks):
                        nc.scalar.activation(
                            h_sb[:, ht, co:co + cs],
                            pss[ci][:, :cs],
                            ACTF.Relu,
                            bias=b1_sb[:, ht:ht + 1],
                        )

                # layer 2: out[c, d] = (h.T @ w2 + b2) * gate, tokens on partitions
                for ti in range(NT):
                    pso = ps2p.tile([P, D], F32, name="ps2", tag="ps2")
                    for ht in range(KH):
                        nc.tensor.matmul(
                            pso[:],
                            lhsT=h_sb[:, ht, ti * P:(ti + 1) * P],
                            rhs=w2_sb[:, ht, :],
                            start=(ht == 0),
                            stop=False,
                        )
                    nc.tensor.matmul(
                        pso[:], lhsT=ones1[:], rhs=b2_sb[:],
                        start=False, stop=True,
                    )
                    o_sb = ob.tile([P, D], F32, name="o", tag="o")
                    nc.vector.tensor_scalar_mul(o_sb[:], pso[:], g_sb[:, ti:ti + 1])
                    nc.sync.dma_start(out[j, ti * P:(ti + 1) * P, :], o_sb[:])
    return _legalize_waits(nc)


def _asf32(a):
    return np.ascontiguousarray(np.asarray(a, dtype=np.float32))


def kernel(x, gate_x, w_gate, w1, b1, w2, b2, k):
    assert int(k) == TOPK
    x = _asf32(x)
    gate_x = _asf32(gate_x)
    w_gate = _asf32(w_gate)
    w1 = _asf32(w1)
    b1 = _asf32(b1)
    w2 = _asf32(w2)
    b2 = _asf32(b2)

    # --- launch 1: gating + aux loss ---
    in1 = [
        {
            "gxT": np.ascontiguousarray(gate_x[c * BSH:(c + 1) * BSH].T),
            "wg": w_gate,
        }
        for c in range(NCORES)
    ]
    r1 = run_bass_kernel_spmd(_gating_nc(), in1, CORE_IDS)
    gates = np.concatenate([r1.results[c]["gates"] for c in range(NCORES)], axis=0)
    loss = np.float32(r1.results[0]["loss"][0, 0])

    # --- host dispatch: gather tokens per expert (index work only) ---
    idxs, gvals = [], []
    for e in range(E):
        idx = np.flatnonzero(gates[:, e])
        idxs.append(idx)
        gvals.append(gates[idx, e])
    maxn = max(len(ix) for ix in idxs)
    C = max(1152, -(-maxn // P) * P)

    in2 = []
    for c in range(NCORES):
        xgT = np.zeros((EPC, D, C), np.float32)
        gv = np.zeros((EPC, C), np.float32)
        for j in range(EPC):
            e = c * EPC + j
            n = len(idxs[e])
            xgT[j, :, :n] = x[idxs[e]].T
            gv[j, :n] = gvals[e]
        in2.append(
            {
                "xgT": xgT,
                "g": gv,
                "ones": np.ones((1, P), np.float32),
                "w1": w1[c * EPC:(c + 1) * EPC],
                "b1": b1[c * EPC:(c + 1) * EPC],
                "w2": w2[c * EPC:(c + 1) * EPC],
                "b2": b2[c * EPC:(c + 1) * EPC],
            }
        )
    r2 = run_bass_kernel_spmd(_expert_nc(C), in2, CORE_IDS)

    # --- host combine: scatter rows back (indices unique per expert) ---
    y = np.zeros((B, D), np.float32)
    for c in range(NCORES):
        o = r2.results[c]["out"]
        for j in range(EPC):
            e = c * EPC + j
            y[idxs[e]] += o[j, : len(idxs[e])]
    return y, loss


# revision 7
# speedup vs baseline: 1.4056x; 1.4056x over previous
"""MoE routing kernel for Trainium2, expert-parallel across 8 NeuronCores.

Strategy (matches the expert-parallel sharding hint):
  Launch 1 (token-parallel gating): each core computes logits = gate_x_shard
    @ w_gate for its 512-token shard, top-4 mask + softmax -> dense sparse
    gates [512, 16], plus per-expert importance/load partials, AllReduce of
    the partials across the 8 cores and the cv^2 aux loss on-device.
  Host dispatch: tokens are gathered per expert from the device-computed
    gates (pure index/data movement), padded to a fixed capacity C.
  Launch 2 (expert-parallel MLP): core c owns experts {2c, 2c+1}. For each:
    h = relu(xg @ w1[e] + b1[e]); out = (h @ w2[e] + b2[e]) * gate, all
    matmuls on the PE array in fp32r (full rate), biases fused (b1 via the
    ACT bias port, b2 via a K=1 ones-row matmul into the same PSUM group).
  Host combine: out rows scatter-add back to token order (unique indices
    per expert, so a plain fancy-index += per expert).
"""

import functools

import numpy as np

import concourse.bass as bass
import concourse.mybir as mybir
from concourse.bass_utils import run_bass_kernel_spmd
from concourse.tile import TileContext
from concourse.vector_clock import ScopedClock, VectorClock

F32 = mybir.dt.float32
F32R = mybir.dt.float32r
AX = mybir.AxisListType
ALU = mybir.AluOpType
ACTF = mybir.ActivationFunctionType

B, D, HD, E, TOPK = 4096, 512, 1024, 16, 4
NCORES = 8
P = 128
BSH = B // NCORES  # 512 gating tokens per core
EPC = E // NCORES  # 2 experts per core
KD = D // P        # 4 contraction slices over D
KH = HD // P       # 8 slices over H
CORE_IDS = list(range(NCORES))


class _TileContextNX(TileContext):
    pass


def _legalize_waits(nc):
    """The AWS neuronxcc CoreV3 codegen accepts only one sync-wait per
    instruction, while Tile attaches all required waits to the consuming
    instruction. Hoist extra waits onto standalone same-engine NoOps placed
    immediately before the instruction (engine queues are FIFO, so ordering
    is preserved)."""
    n = 0
    for f in nc.m.functions:
        for bb in f.blocks:
            new = []
            for inst in bb.instructions:
                si = inst.sync_info
                if si is not None and si.on_wait and len(si.on_wait) > 1:
                    waits = list(si.on_wait)
                    for w in waits[:-1]:
                        n += 1
                        nop = mybir.InstNoOp(
                            name=f"{inst.name}-lw{n}",
                            engine=inst.engine,
                            sync_info=mybir.SyncInfo(on_wait=[w], on_update=[]),
                        )
                        new.append(nop)
                    si.on_wait = [waits[-1]]
                new.append(inst)
            bb.instructions[:] = new
    return nc


def _r(ap):
    """View an fp32 AP as fp32r so the PE runs at full (1 cyc/row) rate."""
    return ap.bitcast(F32R)


@functools.lru_cache(maxsize=None)
def _gating_nc():
    nc = bass.Bass("TRN2", target_bir_lowering=False, debug=False)
    gxT = nc.declare_dram_parameter("gxT", [D, BSH], F32, isOutput=False)
    wg = nc.declare_dram_parameter("wg", [D, E], F32, isOutput=False)
    gates_out = nc.declare_dram_parameter("gates", [BSH, E], F32, isOutput=True)
    part_out = nc.declare_dram_parameter("part", [E, 2], F32, isOutput=True)

    with _TileContextNX(nc) as tc:
        with (
            tc.tile_pool(name="sb", bufs=2) as sb,
            tc.tile_pool(name="acc", bufs=1) as acc,
            tc.tile_pool(name="psl", bufs=2, space="PSUM") as psl,
            tc.tile_pool(name="psa", bufs=1, space="PSUM") as psa,
        ):
            wg_sb = acc.tile([P, KD, E], F32, name="wg", tag="wg")
            nc.sync.dma_start(wg_sb[:], wg.rearrange("(o p) e -> p o e", p=P))
            gx_sb = acc.tile([P, KD, BSH], F32, name="gx", tag="gx")
            for ks in range(KD):
                nc.sync.dma_start(gx_sb[:, ks, :], gxT[ks * P:(ks + 1) * P, :])
            ones_sb = acc.tile([P, 1], F32, name="ones", tag="ones")
            nc.vector.memset(ones_sb[:], 1.0)

            ps_imp = psa.tile([E, 1], F32, name="imp", tag="imp")
            ps_load = psa.tile([E, 1], F32, name="load", tag="load")

            ntt = BSH // P  # 4 token tiles
            for ti in range(ntt):
                ps = psl.tile([P, E], F32, name="logits", tag="logits")
                for ks in range(KD):
                    # full fp32: the top-4 boundary must match the fp32
                    # reference (smallest observed top4/5 gap ~2.5e-6)
                    nc.tensor.matmul(
                        ps[:],
                        lhsT=gx_sb[:, ks, ti * P:(ti + 1) * P],
                        rhs=wg_sb[:, ks, :],
                        start=(ks == 0),
                        stop=(ks == KD - 1),
                    )
                logits = sb.tile([P, E], F32, name="logits_sb", tag="logits_sb")
                nc.vector.tensor_copy(logits[:], ps[:])
                mx = sb.tile([P, 8], F32, name="mx", tag="mx")
                nc.vector.max(mx[:], logits[:])
                neg = sb.tile([P, 1], F32, name="neg", tag="neg")
                nc.vector.tensor_scalar_mul(neg[:], mx[:, 0:1], -1.0)
                expz = sb.tile([P, E], F32, name="expz", tag="expz")
                nc.scalar.activation(expz[:], logits[:], ACTF.Exp, bias=neg[:])
                mask = sb.tile([P, E], F32, name="mask", tag="mask")
                nc.vector.tensor_scalar(
                    mask[:], logits[:], mx[:, 3:4], None, op0=ALU.is_ge
                )
                nc.vector.tensor_mul(expz[:], expz[:], mask[:])
                s = sb.tile([P, 1], F32, name="s", tag="s")
                nc.vector.reduce_sum(s[:], expz[:], axis=AX.X)
                nc.vector.reciprocal(s[:], s[:])
                gates_sb = sb.tile([P, E], F32, name="gates_sb", tag="gates_sb")
                nc.vector.tensor_scalar_mul(gates_sb[:], expz[:], s[:])
                nc.sync.dma_start(gates_out[ti * P:(ti + 1) * P, :], gates_sb[:])

                gtz = sb.tile([P, E], F32, name="gtz", tag="gtz")
                nc.vector.tensor_scalar(
                    gtz[:], gates_sb[:], 0.0, None, op0=ALU.is_gt
                )
                nc.tensor.matmul(
                    ps_imp[:], lhsT=gates_sb[:], rhs=ones_sb[:],
                    start=(ti == 0), stop=(ti == ntt - 1),
                )
                nc.tensor.matmul(
                    ps_load[:], lhsT=gtz[:], rhs=ones_sb[:],
                    start=(ti == 0), stop=(ti == ntt - 1),
                )

            stat_sb = sb.tile([E, 2], F32, name="stat", tag="stat")
            nc.vector.tensor_copy(stat_sb[:, 0:1], ps_imp[:])
            nc.vector.tensor_copy(stat_sb[:, 1:2], ps_load[:])
            nc.sync.dma_start(part_out[:], stat_sb[:])
    return _legalize_waits(nc)


@functools.lru_cache(maxsize=None)
def _expert_nc(C):
    assert C % P == 0
    NT = C // P                      # token tiles per expert
    chunks = []
    off = 0
    while off < C:
        chunks.append((off, min(512, C - off)))
        off += 512

    nc = bass.Bass("TRN2", target_bir_lowering=False, debug=False)
    xgT = nc.declare_dram_parameter("xgT", [EPC, D, C], F32R, isOutput=False)
    w1 = nc.declare_dram_parameter("w1", [EPC, D, HD], F32R, isOutput=False)
    b1 = nc.declare_dram_parameter("b1", [EPC, HD], F32, isOutput=False)
    w2 = nc.declare_dram_parameter("w2", [EPC, HD, D], F32R, isOutput=False)
    b2 = nc.declare_dram_parameter("b2", [EPC, D], F32R, isOutput=False)
    g = nc.declare_dram_parameter("g", [EPC, C], F32, isOutput=False)
    ones = nc.declare_dram_parameter("ones", [1, P], F32R, isOutput=False)
    parts = nc.declare_dram_parameter("parts", [NCORES, E, 2], F32, isOutput=False)
    out = nc.declare_dram_parameter("out", [EPC, C, D], F32, isOutput=True)
    loss_out = nc.declare_dram_parameter("loss", [1, 1], F32, isOutput=True)

    with _TileContextNX(nc) as tc:
        with (
            tc.tile_pool(name="wts", bufs=2) as wts,
            tc.tile_pool(name="xg", bufs=2) as xgp,
            tc.tile_pool(name="hb", bufs=2) as hp,
            tc.tile_pool(name="small", bufs=2) as small,
            tc.tile_pool(name="ob", bufs=3) as ob,
            tc.tile_pool(name="ps1", bufs=2, space="PSUM") as ps1p,
            tc.tile_pool(name="ps2", bufs=2, space="PSUM") as ps2p,
        ):
            ones1 = small.tile([1, P], F32R, name="ones1", tag="ones1")
            nc.sync.dma_start(ones1[:], ones[:])

            # aux loss from the gating launch's per-core partials (cheap,
            # hidden under the first weight DMAs)
            red_sb = small.tile([1, E, 2, NCORES], F32, name="red4", tag="red4")
            nc.sync.dma_start(red_sb[:], parts.rearrange("c e s -> e s c")[None])
            red2 = small.tile([1, E, 2], F32, name="red2", tag="red2")
            nc.vector.reduce_sum(red2[:], red_sb[:], axis=AX.X)

            def _t11(tag):
                return small.tile([1, 1], F32, name=tag, tag=tag)

            cvs = []
            for si in range(2):
                v = red2[:, :, si]  # [1, E]
                sm = _t11(f"sm{si}")
                nc.vector.reduce_sum(sm[:], v, axis=AX.X)
                sq = small.tile([1, E], F32, name=f"sq{si}", tag=f"sq{si}")
                nc.vector.tensor_mul(sq[:], v, v)
                ssq = _t11(f"ssq{si}")
                nc.vector.reduce_sum(ssq[:], sq[:], axis=AX.X)
                mean = _t11(f"mean{si}")
                nc.vector.tensor_scalar_mul(mean[:], sm[:], 1.0 / E)
                m2 = _t11(f"m2{si}")
                nc.vector.tensor_mul(m2[:], mean[:], mean[:])
                nm2 = _t11(f"nm2{si}")
                nc.vector.tensor_scalar_mul(nm2[:], m2[:], -float(E))
                var = _t11(f"var{si}")
                nc.vector.tensor_add(var[:], ssq[:], nm2[:])
                nc.vector.tensor_scalar_mul(var[:], var[:], 1.0 / (E - 1))
                den = _t11(f"den{si}")
                nc.vector.tensor_scalar_add(den[:], m2[:], 1e-10)
                nc.vector.reciprocal(den[:], den[:])
                cv = _t11(f"cv{si}")
                nc.vector.tensor_mul(cv[:], var[:], den[:])
                cvs.append(cv)
            loss_sb = _t11("loss_sb")
            nc.vector.tensor_add(loss_sb[:], cvs[0][:], cvs[1][:])
            nc.sync.dma_start(loss_out[:], loss_sb[:])
            for j in range(EPC):
                w1_sb = wts.tile([P, KD, HD], F32R, name="w1", tag="w1")
                xg_sb = xgp.tile([P, KD, C], F32R, name="xg", tag="xg")
                for ks in range(KD):
                    nc.sync.dma_start(w1_sb[:, ks, :], w1[j, ks * P:(ks + 1) * P, :])
                    nc.sync.dma_start(xg_sb[:, ks, :], xgT[j, ks * P:(ks + 1) * P, :])
                w2_sb = wts.tile([P, KH, D], F32R, name="w2", tag="w2")
                for ht in range(KH):
                    nc.sync.dma_start(w2_sb[:, ht, :], w2[j, ht * P:(ht + 1) * P, :])
                b1_sb = small.tile([P, KH], F32, name="b1", tag="b1")
                nc.sync.dma_start(b1_sb[:], b1[j].rearrange("(o p) -> p o", p=P))
                b2_sb = small.tile([1, D], F32R, name="b2", tag="b2")
                nc.sync.dma_start(b2_sb[:], b2[j][None, :])
                g_sb = small.tile([P, NT], F32, name="g", tag="g")
                nc.sync.dma_start(g_sb[:], g[j].rearrange("(o p) -> p o", p=P))

                # layer 1: hT[h, c] = relu(w1.T @ xg + b1), H on partitions
                h_sb = hp.tile([P, KH, C], F32R, name="h", tag="h")
                for ht in range(KH):
                    pss = [
                        ps1p.tile([P, 512], F32, name=f"ps1_{ci}", tag=f"ps1_{ci}")
                        for ci in range(len(chunks))
                    ]
                    for ks in range(KD):
                        for ci, (co, cs) in enumerate(chunks):
                            nc.tensor.matmul(
                                pss[ci][:, :cs],
                                lhsT=w1_sb[:, ks, ht * P:(ht + 1) * P],
                                rhs=xg_sb[:, ks, co:co + cs],
                                start=(ks == 0),
                                stop=(ks == KD - 1),
                            )
                    for ci, (co, cs) in enumerate(chunks):
                        nc.scalar.activation(
                            h_sb[:, ht, co:co + cs],
                            pss[ci][:, :cs],
                            ACTF.Relu,
                            bias=b1_sb[:, ht:ht + 1],
                        )

                # layer 2: out[c, d] = (h.T @ w2 + b2) * gate, tokens on partitions
                for ti in range(NT):
                    pso = ps2p.tile([P, D], F32, name="ps2", tag="ps2")
                    for ht in range(KH):
                        nc.tensor.matmul(
                            pso[:],
                            lhsT=h_sb[:, ht, ti * P:(ti + 1) * P],
                            rhs=w2_sb[:, ht, :],
                            start=(ht == 0),
                            stop=False,
                        )
                    nc.tensor.matmul(
                        pso[:], lhsT=ones1[:], rhs=b2_sb[:],
                        start=False, stop=True,
                    )
                    o_sb = ob.tile([P, D], F32, name="o", tag="o")
                    nc.vector.tensor_scalar_mul(o_sb[:], pso[:], g_sb[:, ti:ti + 1])
                    nc.sync.dma_start(out[j, ti * P:(ti + 1) * P, :], o_sb[:])
    return _legalize_waits(nc)


def _asf32(a):
    return np.ascontiguousarray(np.asarray(a, dtype=np.float32))


def kernel(x, gate_x, w_gate, w1, b1, w2, b2, k):
    assert int(k) == TOPK
    x = _asf32(x)
    gate_x = _asf32(gate_x)
    w_gate = _asf32(w_gate)
    w1 = _asf32(w1)
    b1 = _asf32(b1)
    w2 = _asf32(w2)
    b2 = _asf32(b2)

    # --- launch 1: gating + aux loss ---
    in1 = [
        {
            "gxT": np.ascontiguousarray(gate_x[c * BSH:(c + 1) * BSH].T),
            "wg": w_gate,
        }
        for c in range(NCORES)
    ]
    r1 = run_bass_kernel_spmd(_gating_nc(), in1, CORE_IDS)
    gates = np.concatenate([r1.results[c]["gates"] for c in range(NCORES)], axis=0)
    parts = np.ascontiguousarray(
        np.stack([r1.results[c]["part"] for c in range(NCORES)], axis=0)
    )

    # --- host dispatch: gather tokens per expert (index work only) ---
    idxs, gvals = [], []
    for e in range(E):
        idx = np.flatnonzero(gates[:, e])
        idxs.append(idx)
        gvals.append(gates[idx, e])
    maxn = max(len(ix) for ix in idxs)
    C = max(1152, -(-maxn // P) * P)

    in2 = []
    for c in range(NCORES):
        xgT = np.zeros((EPC, D, C), np.float32)
        gv = np.zeros((EPC, C), np.float32)
        for j in range(EPC):
            e = c * EPC + j
            n = len(idxs[e])
            xgT[j, :, :n] = x[idxs[e]].T
            gv[j, :n] = gvals[e]
        in2.append(
            {
                "xgT": xgT,
                "g": gv,
                "ones": np.ones((1, P), np.float32),
                "parts": parts,
                "w1": w1[c * EPC:(c + 1) * EPC],
                "b1": b1[c * EPC:(c + 1) * EPC],
                "w2": w2[c * EPC:(c + 1) * EPC],
                "b2": b2[c * EPC:(c + 1) * EPC],
            }
        )
    r2 = run_bass_kernel_spmd(_expert_nc(C), in2, CORE_IDS)
    loss = np.float32(r2.results[0]["loss"][0, 0])

    # --- host combine: scatter rows back (indices unique per expert) ---
    y = np.zeros((B, D), np.float32)
    for c in range(NCORES):
        o = r2.results[c]["out"]
        for j in range(EPC):
            e = c * EPC + j
            y[idxs[e]] += o[j, : len(idxs[e])]
    return y, loss


# revision 8
# speedup vs baseline: 1.6197x; 1.1523x over previous
"""MoE routing kernel for Trainium2, expert-parallel across 8 NeuronCores.

Strategy (matches the expert-parallel sharding hint):
  Launch 1 (token-parallel gating): each core computes logits = gate_x_shard
    @ w_gate for its 512-token shard, top-4 mask + softmax -> dense sparse
    gates [512, 16], plus per-expert importance/load partials, AllReduce of
    the partials across the 8 cores and the cv^2 aux loss on-device.
  Host dispatch: tokens are gathered per expert from the device-computed
    gates (pure index/data movement), padded to a fixed capacity C.
  Launch 2 (expert-parallel MLP): core c owns experts {2c, 2c+1}. For each:
    h = relu(xg @ w1[e] + b1[e]); out = (h @ w2[e] + b2[e]) * gate, all
    matmuls on the PE array in fp32r (full rate), biases fused (b1 via the
    ACT bias port, b2 via a K=1 ones-row matmul into the same PSUM group).
  Host combine: out rows scatter-add back to token order (unique indices
    per expert, so a plain fancy-index += per expert).
"""

import functools

import ml_dtypes
import numpy as np

import concourse.bass as bass
import concourse.mybir as mybir
from concourse.bass_utils import run_bass_kernel_spmd
from concourse.tile import TileContext
from concourse.vector_clock import ScopedClock, VectorClock

F32 = mybir.dt.float32
F32R = mybir.dt.float32r
BF16 = mybir.dt.bfloat16
AX = mybir.AxisListType
ALU = mybir.AluOpType
ACTF = mybir.ActivationFunctionType

B, D, HD, E, TOPK = 4096, 512, 1024, 16, 4
NCORES = 8
P = 128
BSH = B // NCORES  # 512 gating tokens per core
EPC = E // NCORES  # 2 experts per core
KD = D // P        # 4 contraction slices over D
KH = HD // P       # 8 slices over H
CORE_IDS = list(range(NCORES))


class _TileContextNX(TileContext):
    pass


def _legalize_waits(nc):
    """The AWS neuronxcc CoreV3 codegen accepts only one sync-wait per
    instruction, while Tile attaches all required waits to the consuming
    instruction. Hoist extra waits onto standalone same-engine NoOps placed
    immediately before the instruction (engine queues are FIFO, so ordering
    is preserved)."""
    n = 0
    for f in nc.m.functions:
        for bb in f.blocks:
            new = []
            for inst in bb.instructions:
                si = inst.sync_info
                if si is not None and si.on_wait and len(si.on_wait) > 1:
                    waits = list(si.on_wait)
                    for w in waits[:-1]:
                        n += 1
                        nop = mybir.InstNoOp(
                            name=f"{inst.name}-lw{n}",
                            engine=inst.engine,
                            sync_info=mybir.SyncInfo(on_wait=[w], on_update=[]),
                        )
                        new.append(nop)
                    si.on_wait = [waits[-1]]
                new.append(inst)
            bb.instructions[:] = new
    return nc


def _r(ap):
    """View an fp32 AP as fp32r so the PE runs at full (1 cyc/row) rate."""
    return ap.bitcast(F32R)


@functools.lru_cache(maxsize=None)
def _gating_nc():
    nc = bass.Bass("TRN2", target_bir_lowering=False, debug=False)
    gxT = nc.declare_dram_parameter("gxT", [D, BSH], F32, isOutput=False)
    wg = nc.declare_dram_parameter("wg", [D, E], F32, isOutput=False)
    gates_out = nc.declare_dram_parameter("gates", [BSH, E], F32, isOutput=True)
    part_out = nc.declare_dram_parameter("part", [E, 2], F32, isOutput=True)

    with _TileContextNX(nc) as tc:
        with (
            tc.tile_pool(name="sb", bufs=2) as sb,
            tc.tile_pool(name="acc", bufs=1) as acc,
            tc.tile_pool(name="psl", bufs=2, space="PSUM") as psl,
            tc.tile_pool(name="psa", bufs=1, space="PSUM") as psa,
        ):
            wg_sb = acc.tile([P, KD, E], F32, name="wg", tag="wg")
            nc.sync.dma_start(wg_sb[:], wg.rearrange("(o p) e -> p o e", p=P))
            gx_sb = acc.tile([P, KD, BSH], F32, name="gx", tag="gx")
            for ks in range(KD):
                nc.sync.dma_start(gx_sb[:, ks, :], gxT[ks * P:(ks + 1) * P, :])
            ones_sb = acc.tile([P, 1], F32, name="ones", tag="ones")
            nc.vector.memset(ones_sb[:], 1.0)

            ps_imp = psa.tile([E, 1], F32, name="imp", tag="imp")
            ps_load = psa.tile([E, 1], F32, name="load", tag="load")

            ntt = BSH // P  # 4 token tiles
            for ti in range(ntt):
                ps = psl.tile([P, E], F32, name="logits", tag="logits")
                for ks in range(KD):
                    # full fp32: the top-4 boundary must match the fp32
                    # reference (smallest observed top4/5 gap ~2.5e-6)
                    nc.tensor.matmul(
                        ps[:],
                        lhsT=gx_sb[:, ks, ti * P:(ti + 1) * P],
                        rhs=wg_sb[:, ks, :],
                        start=(ks == 0),
                        stop=(ks == KD - 1),
                    )
                logits = sb.tile([P, E], F32, name="logits_sb", tag="logits_sb")
                nc.vector.tensor_copy(logits[:], ps[:])
                mx = sb.tile([P, 8], F32, name="mx", tag="mx")
                nc.vector.max(mx[:], logits[:])
                neg = sb.tile([P, 1], F32, name="neg", tag="neg")
                nc.vector.tensor_scalar_mul(neg[:], mx[:, 0:1], -1.0)
                expz = sb.tile([P, E], F32, name="expz", tag="expz")
                nc.scalar.activation(expz[:], logits[:], ACTF.Exp, bias=neg[:])
                mask = sb.tile([P, E], F32, name="mask", tag="mask")
                nc.vector.tensor_scalar(
                    mask[:], logits[:], mx[:, 3:4], None, op0=ALU.is_ge
                )
                nc.vector.tensor_mul(expz[:], expz[:], mask[:])
                s = sb.tile([P, 1], F32, name="s", tag="s")
                nc.vector.reduce_sum(s[:], expz[:], axis=AX.X)
                nc.vector.reciprocal(s[:], s[:])
                gates_sb = sb.tile([P, E], F32, name="gates_sb", tag="gates_sb")
                nc.vector.tensor_scalar_mul(gates_sb[:], expz[:], s[:])
                nc.sync.dma_start(gates_out[ti * P:(ti + 1) * P, :], gates_sb[:])

                gtz = sb.tile([P, E], F32, name="gtz", tag="gtz")
                nc.vector.tensor_scalar(
                    gtz[:], gates_sb[:], 0.0, None, op0=ALU.is_gt
                )
                nc.tensor.matmul(
                    ps_imp[:], lhsT=gates_sb[:], rhs=ones_sb[:],
                    start=(ti == 0), stop=(ti == ntt - 1),
                )
                nc.tensor.matmul(
                    ps_load[:], lhsT=gtz[:], rhs=ones_sb[:],
                    start=(ti == 0), stop=(ti == ntt - 1),
                )

            stat_sb = sb.tile([E, 2], F32, name="stat", tag="stat")
            nc.vector.tensor_copy(stat_sb[:, 0:1], ps_imp[:])
            nc.vector.tensor_copy(stat_sb[:, 1:2], ps_load[:])
            nc.sync.dma_start(part_out[:], stat_sb[:])
    return _legalize_waits(nc)


@functools.lru_cache(maxsize=None)
def _expert_nc(C):
    assert C % P == 0
    NT = C // P                      # token tiles per expert
    chunks = []
    off = 0
    while off < C:
        chunks.append((off, min(512, C - off)))
        off += 512

    nc = bass.Bass("TRN2", target_bir_lowering=False, debug=False)
    xgT = nc.declare_dram_parameter("xgT", [EPC, D, C], BF16, isOutput=False)
    w1 = nc.declare_dram_parameter("w1", [EPC, D, HD], BF16, isOutput=False)
    b1 = nc.declare_dram_parameter("b1", [EPC, HD], F32, isOutput=False)
    w2 = nc.declare_dram_parameter("w2", [EPC, HD, D], BF16, isOutput=False)
    b2 = nc.declare_dram_parameter("b2", [EPC, D], BF16, isOutput=False)
    g = nc.declare_dram_parameter("g", [EPC, C], F32, isOutput=False)
    ones = nc.declare_dram_parameter("ones", [1, P], BF16, isOutput=False)
    parts = nc.declare_dram_parameter("parts", [NCORES, E, 2], F32, isOutput=False)
    out = nc.declare_dram_parameter("out", [EPC, C, D], F32, isOutput=True)
    loss_out = nc.declare_dram_parameter("loss", [1, 1], F32, isOutput=True)

    with _TileContextNX(nc) as tc:
        with (
            tc.tile_pool(name="wts", bufs=2) as wts,
            tc.tile_pool(name="xg", bufs=2) as xgp,
            tc.tile_pool(name="hb", bufs=2) as hp,
            tc.tile_pool(name="small", bufs=2) as small,
            tc.tile_pool(name="ob", bufs=3) as ob,
            tc.tile_pool(name="ps1", bufs=2, space="PSUM") as ps1p,
            tc.tile_pool(name="ps2", bufs=2, space="PSUM") as ps2p,
        ):
            ones1 = small.tile([1, P], BF16, name="ones1", tag="ones1")
            nc.sync.dma_start(ones1[:], ones[:])

            # aux loss from the gating launch's per-core partials (cheap,
            # hidden under the first weight DMAs)
            red_sb = small.tile([1, E, 2, NCORES], F32, name="red4", tag="red4")
            nc.sync.dma_start(red_sb[:], parts.rearrange("c e s -> e s c")[None])
            red2 = small.tile([1, E, 2], F32, name="red2", tag="red2")
            nc.vector.reduce_sum(red2[:], red_sb[:], axis=AX.X)

            def _t11(tag):
                return small.tile([1, 1], F32, name=tag, tag=tag)

            cvs = []
            for si in range(2):
                v = red2[:, :, si]  # [1, E]
                sm = _t11(f"sm{si}")
                nc.vector.reduce_sum(sm[:], v, axis=AX.X)
                sq = small.tile([1, E], F32, name=f"sq{si}", tag=f"sq{si}")
                nc.vector.tensor_mul(sq[:], v, v)
                ssq = _t11(f"ssq{si}")
                nc.vector.reduce_sum(ssq[:], sq[:], axis=AX.X)
                mean = _t11(f"mean{si}")
                nc.vector.tensor_scalar_mul(mean[:], sm[:], 1.0 / E)
                m2 = _t11(f"m2{si}")
                nc.vector.tensor_mul(m2[:], mean[:], mean[:])
                nm2 = _t11(f"nm2{si}")
                nc.vector.tensor_scalar_mul(nm2[:], m2[:], -float(E))
                var = _t11(f"var{si}")
                nc.vector.tensor_add(var[:], ssq[:], nm2[:])
                nc.vector.tensor_scalar_mul(var[:], var[:], 1.0 / (E - 1))
                den = _t11(f"den{si}")
                nc.vector.tensor_scalar_add(den[:], m2[:], 1e-10)
                nc.vector.reciprocal(den[:], den[:])
                cv = _t11(f"cv{si}")
                nc.vector.tensor_mul(cv[:], var[:], den[:])
                cvs.append(cv)
            loss_sb = _t11("loss_sb")
            nc.vector.tensor_add(loss_sb[:], cvs[0][:], cvs[1][:])
            nc.sync.dma_start(loss_out[:], loss_sb[:])
            for j in range(EPC):
                w1_sb = wts.tile([P, KD, HD], BF16, name="w1", tag="w1")
                xg_sb = xgp.tile([P, KD, C], BF16, name="xg", tag="xg")
                for ks in range(KD):
                    nc.sync.dma_start(w1_sb[:, ks, :], w1[j, ks * P:(ks + 1) * P, :])
                    nc.sync.dma_start(xg_sb[:, ks, :], xgT[j, ks * P:(ks + 1) * P, :])
                w2_sb = wts.tile([P, KH, D], BF16, name="w2", tag="w2")
                for ht in range(KH):
                    nc.sync.dma_start(w2_sb[:, ht, :], w2[j, ht * P:(ht + 1) * P, :])
                b1_sb = small.tile([P, KH], F32, name="b1", tag="b1")
                nc.sync.dma_start(b1_sb[:], b1[j].rearrange("(o p) -> p o", p=P))
                b2_sb = small.tile([1, D], BF16, name="b2", tag="b2")
                nc.sync.dma_start(b2_sb[:], b2[j][None, :])
                g_sb = small.tile([P, NT], F32, name="g", tag="g")
                nc.sync.dma_start(g_sb[:], g[j].rearrange("(o p) -> p o", p=P))

                # layer 1: hT[h, c] = relu(w1.T @ xg + b1), H on partitions
                h_sb = hp.tile([P, KH, C], BF16, name="h", tag="h")
                for ht in range(KH):
                    pss = [
                        ps1p.tile([P, 512], F32, name=f"ps1_{ci}", tag=f"ps1_{ci}")
                        for ci in range(len(chunks))
                    ]
                    for ks in range(KD):
                        for ci, (co, cs) in enumerate(chunks):
                            nc.tensor.matmul(
                                pss[ci][:, :cs],
                                lhsT=w1_sb[:, ks, ht * P:(ht + 1) * P],
                                rhs=xg_sb[:, ks, co:co + cs],
                                start=(ks == 0),
                                stop=(ks == KD - 1),
                            )
                    for ci, (co, cs) in enumerate(chunks):
                        nc.scalar.activation(
                            h_sb[:, ht, co:co + cs],
                            pss[ci][:, :cs],
                            ACTF.Relu,
                            bias=b1_sb[:, ht:ht + 1],
                        )

                # layer 2: out[c, d] = (h.T @ w2 + b2) * gate, tokens on partitions
                for ti in range(NT):
                    pso = ps2p.tile([P, D], F32, name="ps2", tag="ps2")
                    for ht in range(KH):
                        nc.tensor.matmul(
                            pso[:],
                            lhsT=h_sb[:, ht, ti * P:(ti + 1) * P],
                            rhs=w2_sb[:, ht, :],
                            start=(ht == 0),
                            stop=False,
                        )
                    nc.tensor.matmul(
                        pso[:], lhsT=ones1[:], rhs=b2_sb[:],
                        start=False, stop=True,
                    )
                    o_sb = ob.tile([P, D], F32, name="o", tag="o")
                    nc.vector.tensor_scalar_mul(o_sb[:], pso[:], g_sb[:, ti:ti + 1])
                    nc.sync.dma_start(out[j, ti * P:(ti + 1) * P, :], o_sb[:])
    return _legalize_waits(nc)


def _asf32(a):
    return np.ascontiguousarray(np.asarray(a, dtype=np.float32))


def kernel(x, gate_x, w_gate, w1, b1, w2, b2, k):
    assert int(k) == TOPK
    x = _asf32(x)
    gate_x = _asf32(gate_x)
    w_gate = _asf32(w_gate)
    w1 = _asf32(w1)
    b1 = _asf32(b1)
    w2 = _asf32(w2)
    b2 = _asf32(b2)

    # --- launch 1: gating + aux loss ---
    in1 = [
        {
            "gxT": np.ascontiguousarray(gate_x[c * BSH:(c + 1) * BSH].T),
            "wg": w_gate,
        }
        for c in range(NCORES)
    ]
    r1 = run_bass_kernel_spmd(_gating_nc(), in1, CORE_IDS)
    gates = np.concatenate([r1.results[c]["gates"] for c in range(NCORES)], axis=0)
    parts = np.ascontiguousarray(
        np.stack([r1.results[c]["part"] for c in range(NCORES)], axis=0)
    )

    # --- host dispatch: gather tokens per expert (index work only) ---
    idxs, gvals = [], []
    for e in range(E):
        idx = np.flatnonzero(gates[:, e])
        idxs.append(idx)
        gvals.append(gates[idx, e])
    maxn = max(len(ix) for ix in idxs)
    C = max(1152, -(-maxn // P) * P)

    bf16 = ml_dtypes.bfloat16
    x_bf = x.astype(bf16)
    w1_bf = w1.astype(bf16)
    w2_bf = w2.astype(bf16)
    b2_bf = b2.astype(bf16)
    in2 = []
    for c in range(NCORES):
        xgT = np.zeros((EPC, D, C), bf16)
        gv = np.zeros((EPC, C), np.float32)
        for j in range(EPC):
            e = c * EPC + j
            n = len(idxs[e])
            xgT[j, :, :n] = x_bf[idxs[e]].T
            gv[j, :n] = gvals[e]
        in2.append(
            {
                "xgT": xgT,
                "g": gv,
                "ones": np.ones((1, P), bf16),
                "parts": parts,
                "w1": w1_bf[c * EPC:(c + 1) * EPC],
                "b1": b1[c * EPC:(c + 1) * EPC],
                "w2": w2_bf[c * EPC:(c + 1) * EPC],
                "b2": b2_bf[c * EPC:(c + 1) * EPC],
            }
        )
    r2 = run_bass_kernel_spmd(_expert_nc(C), in2, CORE_IDS)
    loss = np.float32(r2.results[0]["loss"][0, 0])

    # --- host combine: scatter rows back (indices unique per expert) ---
    y = np.zeros((B, D), np.float32)
    for c in range(NCORES):
        o = r2.results[c]["out"]
        for j in range(EPC):
            e = c * EPC + j
            y[idxs[e]] += o[j, : len(idxs[e])]
    return y, loss
